# revision 18
# baseline (speedup 1.0000x reference)
"""LGRU Bass/Tile kernel for Trainium2, 8-core data-parallel over batch.

Reference computation (per sequence step t):
    xz = x @ Wz ; xh = x @ Wh                     (input projections)
    z  = sigmoid(xz_t + h @ Uz)
    hc = relu(xh_t + h @ Uh)
    h  = z * h + (1 - z) * hc
Returns all hidden states hs[T, B, H].

Sharding: batch (B=32) split 4-per-core across 8 cores; weights replicated.

Layout/schedule (v4):
  - h lives TRANSPOSED and in fp16 as hsT[128, kc, t*BL+b]; it is both the
    recurrence state (matmul moving operand, no per-step cast; fp16 keeps
    feedback rounding at ~5e-4 total, well under tolerance) and the output
    staging buffer (PE transpose-back upcasts fp16->f32 for free).
  - Recurrence matmuls: U stationary fp16 (FWL), 32 LDW+MM pairs per step.
    PSUM accumulation groups are plane-sequential per bank (PSUM zero
    regions are bank-wide, so groups within a bank must not interleave);
    two banks per step (halves A = H-chunks {0,1}, B = {2,3}), z-gate
    planes first within each half.
  - Per-half tail: az_z -> sigmoid -> w=1-z, q=z*h_prev all complete while
    the PE still runs matmuls; the exposed chain after the last matmul is
    just az_h -> r=max(az_h,0)*w (fused scalar_tensor_tensor) -> h=q+r.
  - The NEXT block's preparation (x DMA, PE transposes of x, projection
    matmuls, PSUM->SBUF copies) is software-pipelined into the current
    block's step loop as small "quanta" emitted between steps, where they
    soak up the exposed-tail stall instead of serializing at block ends.
    The hardware block loop is unrolled 2x so the two x_all staging
    buffers alternate by block parity; x is padded by one block on the
    host so the final prefetch reads zeros instead of running off the end.
  - Input projections stay in bf16 hi+lo (3 matmuls per F-chunk) for
    accuracy; their PSUM->SBUF copies ride the mostly idle ACT engine.
"""

import os

import numpy as np

T, B, F, H = 2048, 32, 256, 512
NCORES = 8
BL = B // NCORES  # batch per core = 4
TBLK = 128  # timesteps per block
KC = H // 128  # 4 H-chunks
FC = F // 128  # 2 F-chunks
PT = (TBLK * BL) // 128  # 4 partition-tiles of (t,b) rows per block

_CACHED = {}


def _build_nc(t_total, hilo=True, repeat=1):
    import concourse.bass as bass
    import concourse.mybir as mybir
    from concourse import bacc
    import concourse.tile as tile
    from concourse.bass import ds
    from concourse.masks import make_identity

    FD = mybir.dt.float32
    BF = mybir.dt.bfloat16
    HF = mybir.dt.float16
    nblk = t_total // TBLK
    assert nblk % 2 == 0

    nc = bacc.Bacc("TRN2", target_bir_lowering=False, debug=False)
    # one extra zero block so the last iteration's prefetch stays in bounds
    x = nc.dram_tensor("x", [t_total + TBLK, BL, F], FD, kind="ExternalInput")
    Wz = nc.dram_tensor("Wz", [F, H], FD, kind="ExternalInput")
    Wh = nc.dram_tensor("Wh", [F, H], FD, kind="ExternalInput")
    Uz = nc.dram_tensor("Uz", [H, H], FD, kind="ExternalInput")
    Uh = nc.dram_tensor("Uh", [H, H], FD, kind="ExternalInput")
    hs = nc.dram_tensor("hs", [t_total, BL, H], FD, kind="ExternalOutput")

    x_flat = x.rearrange("t b f -> (t b) f")
    hs_flat = hs.rearrange("t b h -> (t b) h")

    Sig = mybir.ActivationFunctionType.Sigmoid

    with tile.TileContext(nc) as tc:
        with (
            tc.tile_pool(name="const", bufs=1) as constp,
            tc.tile_pool(name="setup", bufs=2) as setupp,
            tc.tile_pool(name="state", bufs=1) as statep,
            tc.tile_pool(name="xblk", bufs=2) as xblkp,
            tc.tile_pool(name="work", bufs=3) as workp,
            tc.tile_pool(name="step", bufs=3) as stepp,
            tc.tile_pool(name="ps_ra", bufs=2, space="PSUM") as ps_ra,
            tc.tile_pool(name="ps_rb", bufs=2, space="PSUM") as ps_rb,
            tc.tile_pool(name="ps_big", bufs=2, space="PSUM") as ps_big,
            tc.tile_pool(name="ps_tr", bufs=2, space="PSUM") as ps_tr,
        ):
            ident = constp.tile([128, 128], FD, tag="ident")
            make_identity(nc, ident)
            ident_hf = constp.tile([128, 128], HF, tag="ident_hf")
            nc.vector.tensor_copy(ident_hf, ident)

            # --- U blocks, fp16 single precision ---
            Ub = {}
            for g, Usrc in (("z", Uz), ("h", Uh)):
                for kc in range(KC):
                    stage = setupp.tile(
                        [128, H], FD, tag=f"stage{g}{kc}", name=f"stage{g}{kc}"
                    )
                    nc.sync.dma_start(out=stage, in_=Usrc[kc * 128 : (kc + 1) * 128, :])
                    ub = constp.tile([128, H], HF, tag=f"U{g}{kc}")
                    nc.vector.tensor_copy(ub, stage)
                    Ub[(g, kc)] = ub

            # --- W blocks, bf16 hi+lo: Wcat = [Wz | Wh] along output dim ---
            Wb = []
            Wb_lo = []
            for kc in range(FC):
                wtile = constp.tile([128, 2 * H], BF, tag=f"W{kc}")
                wlo = constp.tile([128, 2 * H], BF, tag=f"Wl{kc}", name=f"Wl{kc}")
                for si, Wsrc in enumerate((Wz, Wh)):
                    stage = setupp.tile(
                        [128, H], FD, tag=f"stageW{kc}{si}", name=f"stageW{kc}{si}"
                    )
                    nc.sync.dma_start(out=stage, in_=Wsrc[kc * 128 : (kc + 1) * 128, :])
                    nc.vector.tensor_copy(wtile[:, si * H : (si + 1) * H], stage)
                    nc.vector.tensor_sub(
                        wlo[:, si * H : (si + 1) * H],
                        stage,
                        wtile[:, si * H : (si + 1) * H],
                    )
                Wb.append(wtile)
                Wb_lo.append(wlo)

            # --- persistent state: transposed h states, fp16 ---
            hsT = statep.tile([128, KC, TBLK * BL], HF, tag="hsT")
            nc.vector.memset(hsT[:, :, (TBLK - 1) * BL :], 0.0)

            # x_all staging buffers, one per block parity.
            # plane order: [z0 z1 h0 h1 | z2 z3 h2 h3] (mt pairs per half)
            x_allP = [
                statep.tile([128, 8, TBLK * BL], FD, name=f"x_all{p}", tag=f"xa{p}")
                for p in range(2)
            ]

            def plane_of(g, mt):
                return (mt // 2) * 4 + (0 if g == "z" else 2) + (mt % 2)

            def make_prep_quanta(row0_expr, x_all_dst, pfx):
                """Emit-closures preparing x_all_dst for the block at
                row0_expr. Each closure emits one small batch of engine ops;
                they are spread across the previous block's steps."""
                xT = [
                    xblkp.tile([128, TBLK * BL], BF, tag=f"xT{fc}", name=f"{pfx}xT{fc}")
                    for fc in range(FC)
                ]
                xT_lo = [
                    xblkp.tile(
                        [128, TBLK * BL], BF, tag=f"xTl{fc}", name=f"{pfx}xTl{fc}"
                    )
                    for fc in range(FC)
                ]
                xins = [
                    workp.tile([128, F], FD, tag="xin", bufs=4, name=f"{pfx}xin{pt}")
                    for pt in range(PT)
                ]
                quanta = []
                for pt in range(PT):
                    def dma_q(pt=pt):
                        nc.sync.dma_start(
                            out=xins[pt], in_=x_flat[ds(row0_expr + pt * 128, 128), :]
                        )
                    quanta.append(dma_q)
                for pt in range(PT):
                    for fc in range(FC):
                        def tr_q(pt=pt, fc=fc):
                            pst = ps_tr.tile([128, 128], FD, tag="tr", bufs=1)
                            nc.tensor.transpose(
                                pst, xins[pt][:, fc * 128 : (fc + 1) * 128], ident
                            )
                            sl = slice(pt * 128, (pt + 1) * 128)
                            nc.scalar.copy(xT[fc][:, sl], pst)
                            if hilo:
                                nc.vector.tensor_sub(
                                    xT_lo[fc][:, sl], pst, xT[fc][:, sl]
                                )
                        quanta.append(tr_q)
                for g, Wcol in (("z", 0), ("h", 1)):
                    for mt in range(KC):
                        psp = ps_big.tile(
                            [128, TBLK * BL], FD, tag="proj", name=f"{pfx}pj{g}{mt}"
                        )
                        lhs_sl = slice(Wcol * H + mt * 128, Wcol * H + (mt + 1) * 128)
                        terms = []
                        for kc in range(FC):
                            terms.append((Wb[kc][:, lhs_sl], xT[kc]))
                            if hilo:
                                terms.append((Wb_lo[kc][:, lhs_sl], xT[kc]))
                                terms.append((Wb[kc][:, lhs_sl], xT_lo[kc]))
                        n = len(terms)
                        for i, (lhsT_ap, rhs_ap) in enumerate(terms):
                            def mm_q(lhsT_ap=lhsT_ap, rhs_ap=rhs_ap, i=i, n=n, psp=psp):
                                nc.tensor.matmul(
                                    psp,
                                    lhsT=lhsT_ap,
                                    rhs=rhs_ap,
                                    start=(i == 0),
                                    stop=(i == n - 1),
                                )
                            quanta.append(mm_q)
                        def cp_q(psp=psp, plane=plane_of(g, mt)):
                            nc.scalar.copy(x_all_dst[:, plane, :], psp)
                        quanta.append(cp_q)
                return quanta

            def emit_step(tp, x_all_cur):
                cur = ds(tp * BL, BL)
                prev = ds((tp - 1) * BL, BL) if tp > 0 else ds((TBLK - 1) * BL, BL)
                for half, ps_pool in ((0, ps_ra), (1, ps_rb)):
                    ps_t = ps_pool.tile([128, 4, BL], FD, tag=f"ps{half}")
                    groups = [
                        ("z", 2 * half),
                        ("z", 2 * half + 1),
                        ("h", 2 * half),
                        ("h", 2 * half + 1),
                    ]
                    for li, (g, mt) in enumerate(groups):
                        for kc in range(KC):
                            nc.tensor.matmul(
                                ps_t[:, li, :],
                                lhsT=Ub[(g, kc)][:, mt * 128 : (mt + 1) * 128],
                                rhs=hsT[:, kc, prev],
                                start=(kc == 0),
                                stop=(kc == KC - 1),
                            )
                    csl = slice(2 * half, 2 * half + 2)
                    hn = "A" if half == 0 else "B"
                    Mul = mybir.AluOpType.mult
                    Add = mybir.AluOpType.add
                    azz = stepp.tile([128, 2, BL], FD, tag=f"azz{hn}")
                    nc.vector.scalar_tensor_tensor(
                        azz, ps_t[:, 0:2, :], 1.0,
                        x_all_cur[:, 4 * half : 4 * half + 2, cur], Mul, Add,
                    )
                    z = stepp.tile([128, 2, BL], FD, tag=f"z{hn}")
                    nc.scalar.activation(z, azz, Sig)
                    w = stepp.tile([128, 2, BL], FD, tag=f"w{hn}")
                    nc.vector.tensor_scalar(w, z, -1.0, 1.0, Mul, Add)
                    q = stepp.tile([128, 2, BL], FD, tag=f"q{hn}")
                    nc.vector.scalar_tensor_tensor(
                        q, z, 1.0, hsT[:, csl, prev], Mul, Mul
                    )
                    azh = stepp.tile([128, 2, BL], FD, tag=f"azh{hn}")
                    nc.vector.scalar_tensor_tensor(
                        azh, ps_t[:, 2:4, :], 1.0,
                        x_all_cur[:, 4 * half + 2 : 4 * half + 4, cur], Mul, Add,
                    )
                    r = stepp.tile([128, 2, BL], FD, tag=f"r{hn}")
                    nc.vector.scalar_tensor_tensor(
                        r, azh, 0.0, w, mybir.AluOpType.max, Mul
                    )
                    nc.vector.scalar_tensor_tensor(
                        hsT[:, csl, cur], r, 1.0, q, Mul, Add
                    )

            def emit_out_pt(row0_expr, ct):
                # transpose PT-chunk ct back to natural layout (upcast), store
                hnat = workp.tile([128, H], FD, tag="hnat", bufs=4)
                for c in range(KC):
                    pst = ps_tr.tile([128, 128], HF, tag="trb", bufs=1)
                    nc.tensor.transpose(
                        pst, hsT[:, c, ct * 128 : (ct + 1) * 128], ident_hf
                    )
                    nc.scalar.copy(hnat[:, c * 128 : (c + 1) * 128], pst)
                nc.sync.dma_start(
                    out=hs_flat[ds(row0_expr + ct * 128, 128), :], in_=hnat
                )

            def emit_block(row0_expr, x_all_cur, quanta):
                # out-transposes for PT-chunk ct trickle in once its last
                # step (32*(ct+1)-1) has run; the final chunk lands at the end.
                nq = len(quanta)
                qi = 0
                steps_per_pt = TBLK // PT
                for tp in range(TBLK):
                    emit_step(tp, x_all_cur)
                    tgt = (tp + 1) * nq // TBLK
                    while qi < tgt:
                        quanta[qi]()
                        qi += 1
                    if (tp + 1) % steps_per_pt == 0 and tp + 1 < TBLK:
                        emit_out_pt(row0_expr, (tp + 1) // steps_per_pt - 1)
                emit_out_pt(row0_expr, PT - 1)

            import contextlib

            rep_cm = (
                tc.For_i(0, repeat, 1, name="repl")
                if repeat > 1
                else contextlib.nullcontext()
            )
            with rep_cm:
                # prologue: prepare block 0 (no steps to hide under)
                for q in make_prep_quanta(0, x_allP[0], "pro"):
                    q()
                RB = 2 * TBLK * BL  # rows per unrolled pair
                with tc.For_i(0, nblk // 2, 1, staggered_reset=True) as j:
                    r_even = j * RB
                    emit_block(
                        r_even, x_allP[0], make_prep_quanta(r_even + TBLK * BL, x_allP[1], "e")
                    )
                    r_odd = j * RB + TBLK * BL
                    emit_block(
                        r_odd, x_allP[1], make_prep_quanta(r_odd + TBLK * BL, x_allP[0], "o")
                    )

    nc.finalize()
    return nc


def _make_runner(nc, n_cores):
    """Build a cached jitted executor for nc (compile once, reuse across
    kernel() calls). Mirrors bass2jax.run_bass_via_pjrt's multi-core path."""
    import jax
    from jax.sharding import Mesh, NamedSharding, PartitionSpec
    from jax.experimental.shard_map import shard_map
    from concourse import bass2jax, mybir
    from concourse.bass2jax import _bass_exec_p, partition_id_tensor

    bass2jax.install_neuronx_cc_hook()
    partition_name = nc.partition_id_tensor.name if nc.partition_id_tensor else None
    in_names, out_names, out_avals, zero_shapes = [], [], [], []
    for alloc in nc.m.functions[0].allocations:
        if not isinstance(alloc, mybir.MemoryLocationSet):
            continue
        name = alloc.memorylocations[0].name
        if alloc.kind == "ExternalInput":
            if name != partition_name:
                in_names.append(name)
        elif alloc.kind == "ExternalOutput":
            out_names.append(name)
            shape = tuple(alloc.tensor_shape)
            dtype = mybir.dt.np(alloc.dtype)
            out_avals.append(jax.core.ShapedArray(shape, dtype))
            zero_shapes.append((shape, dtype))
    n_params = len(in_names)
    n_outs = len(out_avals)
    all_in_names = list(in_names) + out_names
    if partition_name is not None:
        all_in_names.append(partition_name)
    donate = tuple(range(n_params, n_params + n_outs))

    def _body(*args_):
        operands = list(args_)
        if partition_name is not None:
            operands.append(partition_id_tensor())
        outs = _bass_exec_p.bind(
            *operands,
            out_avals=tuple(out_avals),
            in_names=tuple(all_in_names),
            out_names=tuple(out_names),
            lowering_input_output_aliases=(),
            sim_require_finite=True,
            sim_require_nnan=True,
            nc=nc,
        )
        return tuple(outs)

    devices = jax.devices()[:n_cores]
    mesh = Mesh(np.asarray(devices), ("core",))
    in_specs = (PartitionSpec("core"),) * (n_params + n_outs)
    out_specs = (PartitionSpec("core"),) * len(out_names)
    sharded = jax.jit(
        shard_map(
            _body, mesh=mesh, in_specs=in_specs, out_specs=out_specs, check_rep=False
        ),
        donate_argnums=donate,
        keep_unused=True,
    )
    sh = NamedSharding(mesh, PartitionSpec("core"))

    def run(in_maps):
        per_core = [[np.asarray(m[n]) for n in in_names] for m in in_maps]
        concat_in = [
            np.concatenate([per_core[c][i] for c in range(n_cores)], axis=0)
            for i in range(n_params)
        ]
        dev_in = [jax.device_put(a, sh) for a in concat_in]
        zs = [
            jax.device_put(np.zeros((n_cores * s[0], *s[1:]), d), sh)
            for (s, d) in zero_shapes
        ]
        outs = sharded(*dev_in, *zs)
        return [
            {
                name: np.asarray(outs[i]).reshape(n_cores, *out_avals[i].shape)[c]
                for i, name in enumerate(out_names)
            }
            for c in range(n_cores)
        ]

    return run


def kernel(x, Wz, Wh, Uz, Uh):
    from concourse.bass_utils import run_bass_kernel_spmd

    t_total = x.shape[0]
    hilo = os.environ.get("LGRU_HILO", "1") == "1"
    key = (t_total, hilo)
    if key not in _CACHED:
        _CACHED[key] = _build_nc(t_total, hilo=hilo)
    nc = _CACHED[key]

    x = np.ascontiguousarray(np.asarray(x, dtype=np.float32))
    Wz = np.ascontiguousarray(np.asarray(Wz, dtype=np.float32))
    Wh = np.ascontiguousarray(np.asarray(Wh, dtype=np.float32))
    Uz = np.ascontiguousarray(np.asarray(Uz, dtype=np.float32))
    Uh = np.ascontiguousarray(np.asarray(Uh, dtype=np.float32))

    xpad = np.zeros((t_total + TBLK, x.shape[1], x.shape[2]), np.float32)
    xpad[:t_total] = x

    in_maps = []
    for c in range(NCORES):
        in_maps.append(
            {
                "x": np.ascontiguousarray(xpad[:, c * BL : (c + 1) * BL, :]),
                "Wz": Wz,
                "Wh": Wh,
                "Uz": Uz,
                "Uh": Uh,
            }
        )

    trace = os.environ.get("LGRU_TRACE", "0") == "1"
    if trace:
        # traced path (works only where the NTFF hook exists)
        try:
            res = run_bass_kernel_spmd(
                nc, in_maps, core_ids=list(range(NCORES)), trace=True
            )
            if res.exec_time_ns is not None:
                print(f"HW exec time: {res.exec_time_ns} ns")
                kernel.last_exec_time_ns = res.exec_time_ns
                kernel.last_trace = res.instructions_and_trace
            return np.concatenate([r["hs"] for r in res.results], axis=1)
        except (ImportError, ModuleNotFoundError):
            pass
    rkey = ("runner", key)
    if rkey not in _CACHED:
        _CACHED[rkey] = _make_runner(nc, NCORES)
    results = _CACHED[rkey](in_maps)
    out = np.concatenate([r["hs"] for r in results], axis=1)
    return out


# revision 28
# speedup vs baseline: 1.1407x; 1.1407x over previous
"""LGRU Bass/Tile kernel for Trainium2, 8-core data-parallel over batch.

Reference computation (per sequence step t):
    xz = x @ Wz ; xh = x @ Wh                     (input projections)
    z  = sigmoid(xz_t + h @ Uz)
    hc = relu(xh_t + h @ Uh)
    h  = z * h + (1 - z) * hc
Returns all hidden states hs[T, B, H].

Sharding: batch (B=32) split 4-per-core across 8 cores; weights replicated.

Layout/schedule (v4):
  - h lives TRANSPOSED and in fp16 as hsT[128, kc, t*BL+b]; it is both the
    recurrence state (matmul moving operand, no per-step cast; fp16 keeps
    feedback rounding at ~5e-4 total, well under tolerance) and the output
    staging buffer (PE transpose-back upcasts fp16->f32 for free).
  - Recurrence matmuls: U stationary fp16 (FWL), 32 LDW+MM pairs per step.
    PSUM accumulation groups are plane-sequential per bank (PSUM zero
    regions are bank-wide, so groups within a bank must not interleave);
    two banks per step (halves A = H-chunks {0,1}, B = {2,3}), z-gate
    planes first within each half.
  - Per-half tail: az_z -> sigmoid -> w=1-z, q=z*h_prev all complete while
    the PE still runs matmuls; the exposed chain after the last matmul is
    just az_h -> r=max(az_h,0)*w (fused scalar_tensor_tensor) -> h=q+r.
  - The NEXT block's preparation (x DMA, PE transposes of x, projection
    matmuls, PSUM->SBUF copies) is software-pipelined into the current
    block's step loop as small "quanta" emitted between steps, where they
    soak up the exposed-tail stall instead of serializing at block ends.
    The hardware block loop is unrolled 2x so the two x_all staging
    buffers alternate by block parity; x is padded by one block on the
    host so the final prefetch reads zeros instead of running off the end.
  - Input projections stay in bf16 hi+lo (3 matmuls per F-chunk) for
    accuracy; their PSUM->SBUF copies ride the mostly idle ACT engine.
"""

import os

import numpy as np

T, B, F, H = 2048, 32, 256, 512
NCORES = 8
BL = B // NCORES  # batch per core = 4
TBLK = 128  # timesteps per block
KC = H // 128  # 4 H-chunks
FC = F // 128  # 2 F-chunks
PT = (TBLK * BL) // 128  # 4 partition-tiles of (t,b) rows per block

_CACHED = {}


def _build_nc(t_total, hilo=True, repeat=1):
    import concourse.bass as bass
    import concourse.mybir as mybir
    from concourse import bacc
    import concourse.tile as tile
    from concourse.bass import ds
    from concourse.masks import make_identity

    FD = mybir.dt.float32
    BF = mybir.dt.bfloat16
    HF = mybir.dt.float16
    nblk = t_total // TBLK
    assert nblk % 2 == 0

    nc = bacc.Bacc("TRN2", target_bir_lowering=False, debug=False)
    # one extra zero block so the last iteration's prefetch stays in bounds
    x = nc.dram_tensor("x", [t_total + TBLK, BL, F], FD, kind="ExternalInput")
    Wz = nc.dram_tensor("Wz", [F, H], FD, kind="ExternalInput")
    Wh = nc.dram_tensor("Wh", [F, H], FD, kind="ExternalInput")
    Uz = nc.dram_tensor("Uz", [H, H], FD, kind="ExternalInput")
    Uh = nc.dram_tensor("Uh", [H, H], FD, kind="ExternalInput")
    hs = nc.dram_tensor("hs", [t_total, BL, H], FD, kind="ExternalOutput")

    x_flat = x.rearrange("t b f -> (t b) f")
    hs_flat = hs.rearrange("t b h -> (t b) h")

    Sig = mybir.ActivationFunctionType.Sigmoid

    with tile.TileContext(nc) as tc:
        with (
            tc.tile_pool(name="const", bufs=1) as constp,
            tc.tile_pool(name="setup", bufs=2) as setupp,
            tc.tile_pool(name="state", bufs=1) as statep,
            tc.tile_pool(name="xblk", bufs=2) as xblkp,
            tc.tile_pool(name="work", bufs=3) as workp,
            tc.tile_pool(name="step", bufs=3) as stepp,
            tc.tile_pool(name="ps_ra", bufs=2, space="PSUM") as ps_ra,
            tc.tile_pool(name="ps_rb", bufs=2, space="PSUM") as ps_rb,
            tc.tile_pool(name="ps_big", bufs=2, space="PSUM") as ps_big,
            tc.tile_pool(name="ps_tr", bufs=2, space="PSUM") as ps_tr,
        ):
            ident = constp.tile([128, 128], FD, tag="ident")
            make_identity(nc, ident)
            ident_hf = constp.tile([128, 128], HF, tag="ident_hf")
            nc.vector.tensor_copy(ident_hf, ident)

            # --- U blocks, fp16 single precision ---
            Ub = {}
            for g, Usrc in (("z", Uz), ("h", Uh)):
                for kc in range(KC):
                    stage = setupp.tile(
                        [128, H], FD, tag=f"stage{g}{kc}", name=f"stage{g}{kc}"
                    )
                    nc.sync.dma_start(out=stage, in_=Usrc[kc * 128 : (kc + 1) * 128, :])
                    ub = constp.tile([128, H], HF, tag=f"U{g}{kc}")
                    nc.vector.tensor_copy(ub, stage)
                    Ub[(g, kc)] = ub

            # --- W blocks, bf16 hi+lo: Wcat = [Wz | Wh] along output dim ---
            Wb = []
            Wb_lo = []
            for kc in range(FC):
                wtile = constp.tile([128, 2 * H], BF, tag=f"W{kc}")
                wlo = constp.tile([128, 2 * H], BF, tag=f"Wl{kc}", name=f"Wl{kc}")
                for si, Wsrc in enumerate((Wz, Wh)):
                    stage = setupp.tile(
                        [128, H], FD, tag=f"stageW{kc}{si}", name=f"stageW{kc}{si}"
                    )
                    nc.sync.dma_start(out=stage, in_=Wsrc[kc * 128 : (kc + 1) * 128, :])
                    nc.vector.tensor_copy(wtile[:, si * H : (si + 1) * H], stage)
                    nc.vector.tensor_sub(
                        wlo[:, si * H : (si + 1) * H],
                        stage,
                        wtile[:, si * H : (si + 1) * H],
                    )
                Wb.append(wtile)
                Wb_lo.append(wlo)

            # --- persistent state: transposed h states, fp16 ---
            hsT = statep.tile([128, KC, TBLK * BL], HF, tag="hsT")
            nc.vector.memset(hsT[:, :, (TBLK - 1) * BL :], 0.0)

            # x_all staging buffers, one per block parity.
            # plane order: [z0 z1 h0 h1 | z2 z3 h2 h3] (mt pairs per half)
            x_allP = [
                statep.tile([128, 8, TBLK * BL], FD, name=f"x_all{p}", tag=f"xa{p}")
                for p in range(2)
            ]

            def plane_of(g, mt):
                return (mt // 2) * 4 + (0 if g == "z" else 2) + (mt % 2)

            def make_prep_quanta(row0_expr, x_all_dst, pfx):
                """Emit-closures preparing x_all_dst for the block at
                row0_expr. Each closure emits one small batch of engine ops;
                they are spread across the previous block's steps."""
                xT = [
                    xblkp.tile([128, TBLK * BL], BF, tag=f"xT{fc}", name=f"{pfx}xT{fc}")
                    for fc in range(FC)
                ]
                xT_lo = [
                    xblkp.tile(
                        [128, TBLK * BL], BF, tag=f"xTl{fc}", name=f"{pfx}xTl{fc}"
                    )
                    for fc in range(FC)
                ]
                xins = [
                    workp.tile([128, F], FD, tag="xin", bufs=4, name=f"{pfx}xin{pt}")
                    for pt in range(PT)
                ]
                quanta = []
                for pt in range(PT):
                    def dma_q(pt=pt):
                        nc.sync.dma_start(
                            out=xins[pt], in_=x_flat[ds(row0_expr + pt * 128, 128), :]
                        )
                    quanta.append(dma_q)
                for pt in range(PT):
                    for fc in range(FC):
                        def tr_q(pt=pt, fc=fc):
                            pst = ps_tr.tile([128, 128], FD, tag="tr", bufs=1)
                            nc.tensor.transpose(
                                pst, xins[pt][:, fc * 128 : (fc + 1) * 128], ident
                            )
                            sl = slice(pt * 128, (pt + 1) * 128)
                            nc.scalar.copy(xT[fc][:, sl], pst)
                            if hilo:
                                nc.vector.tensor_sub(
                                    xT_lo[fc][:, sl], pst, xT[fc][:, sl]
                                )
                        quanta.append(tr_q)
                for g, Wcol in (("z", 0), ("h", 1)):
                    for mt in range(KC):
                        psp = ps_big.tile(
                            [128, TBLK * BL], FD, tag="proj", name=f"{pfx}pj{g}{mt}"
                        )
                        lhs_sl = slice(Wcol * H + mt * 128, Wcol * H + (mt + 1) * 128)
                        terms = []
                        for kc in range(FC):
                            terms.append((Wb[kc][:, lhs_sl], xT[kc]))
                            if hilo:
                                terms.append((Wb_lo[kc][:, lhs_sl], xT[kc]))
                                terms.append((Wb[kc][:, lhs_sl], xT_lo[kc]))
                        n = len(terms)
                        for i, (lhsT_ap, rhs_ap) in enumerate(terms):
                            def mm_q(lhsT_ap=lhsT_ap, rhs_ap=rhs_ap, i=i, n=n, psp=psp):
                                nc.tensor.matmul(
                                    psp,
                                    lhsT=lhsT_ap,
                                    rhs=rhs_ap,
                                    start=(i == 0),
                                    stop=(i == n - 1),
                                )
                            quanta.append(mm_q)
                        def cp_q(psp=psp, plane=plane_of(g, mt)):
                            nc.scalar.copy(x_all_dst[:, plane, :], psp)
                        quanta.append(cp_q)
                return quanta

            def emit_step(tp, x_all_cur):
                cur = ds(tp * BL, BL)
                prev = ds((tp - 1) * BL, BL) if tp > 0 else ds((TBLK - 1) * BL, BL)
                for half, ps_pool in ((0, ps_ra), (1, ps_rb)):
                    hn = "A" if half == 0 else "B"
                    ps_t = ps_pool.tile([128, 4, BL], FD, tag=f"ps{half}")
                    # z-gate planes first: sigmoid/w/q run while the PE still
                    # issues the h-plane matmuls.  Accumulation groups are
                    # strictly sequential within the bank (PSUM zero regions).
                    groups = [
                        ("z", 2 * half),
                        ("z", 2 * half + 1),
                        ("h", 2 * half),
                        ("h", 2 * half + 1),
                    ]
                    for li, (g, mt) in enumerate(groups):
                        for kc in range(KC):
                            nc.tensor.matmul(
                                ps_t[:, li, :],
                                lhsT=Ub[(g, kc)][:, mt * 128 : (mt + 1) * 128],
                                rhs=hsT[:, kc, prev],
                                start=(kc == 0),
                                stop=(kc == KC - 1),
                            )
                    csl = slice(2 * half, 2 * half + 2)
                    Mul = mybir.AluOpType.mult
                    Add = mybir.AluOpType.add
                    azz = stepp.tile([128, 2, BL], FD, tag=f"azz{hn}")
                    nc.vector.scalar_tensor_tensor(
                        azz, ps_t[:, 0:2, :], 1.0,
                        x_all_cur[:, 4 * half : 4 * half + 2, cur], Mul, Add,
                    )
                    z = stepp.tile([128, 2, BL], FD, tag=f"z{hn}")
                    nc.scalar.activation(z, azz, Sig)
                    w = stepp.tile([128, 2, BL], FD, tag=f"w{hn}")
                    nc.vector.tensor_scalar(w, z, -1.0, 1.0, Mul, Add)
                    q = stepp.tile([128, 2, BL], FD, tag=f"q{hn}")
                    nc.vector.scalar_tensor_tensor(
                        q, z, 1.0, hsT[:, csl, prev], Mul, Mul
                    )
                    azh = stepp.tile([128, 2, BL], FD, tag=f"azh{hn}")
                    nc.vector.scalar_tensor_tensor(
                        azh, ps_t[:, 2:4, :], 1.0,
                        x_all_cur[:, 4 * half + 2 : 4 * half + 4, cur], Mul, Add,
                    )
                    r = stepp.tile([128, 2, BL], FD, tag=f"r{hn}")
                    nc.vector.scalar_tensor_tensor(
                        r, azh, 0.0, w, mybir.AluOpType.max, Mul
                    )
                    nc.vector.scalar_tensor_tensor(
                        hsT[:, csl, cur], r, 1.0, q, Mul, Add
                    )

            def emit_out_pt(row0_expr, ct):
                # transpose PT-chunk ct back to natural layout (upcast), store
                hnat = workp.tile([128, H], FD, tag="hnat", bufs=4)
                for c in range(KC):
                    pst = ps_tr.tile([128, 128], HF, tag="trb", bufs=1)
                    nc.tensor.transpose(
                        pst, hsT[:, c, ct * 128 : (ct + 1) * 128], ident_hf
                    )
                    nc.scalar.copy(hnat[:, c * 128 : (c + 1) * 128], pst)
                nc.sync.dma_start(
                    out=hs_flat[ds(row0_expr + ct * 128, 128), :], in_=hnat
                )

            def emit_block(row0_expr, x_all_cur, quanta):
                # out-transposes for PT-chunk ct trickle in once its last
                # step (32*(ct+1)-1) has run; the final chunk lands at the end.
                nq = len(quanta)
                qi = 0
                steps_per_pt = TBLK // PT
                for tp in range(TBLK):
                    emit_step(tp, x_all_cur)
                    tgt = (tp + 1) * nq // TBLK
                    while qi < tgt:
                        quanta[qi]()
                        qi += 1
                    if (tp + 1) % steps_per_pt == 0 and tp + 1 < TBLK:
                        emit_out_pt(row0_expr, (tp + 1) // steps_per_pt - 1)
                emit_out_pt(row0_expr, PT - 1)

            import contextlib

            rep_cm = (
                tc.For_i(0, repeat, 1, name="repl")
                if repeat > 1
                else contextlib.nullcontext()
            )
            with rep_cm:
                # prologue: prepare block 0 (no steps to hide under)
                for q in make_prep_quanta(0, x_allP[0], "pro"):
                    q()
                RB = 2 * TBLK * BL  # rows per unrolled pair
                with tc.For_i(0, nblk // 2, 1, staggered_reset=True) as j:
                    r_even = j * RB
                    emit_block(
                        r_even, x_allP[0], make_prep_quanta(r_even + TBLK * BL, x_allP[1], "e")
                    )
                    r_odd = j * RB + TBLK * BL
                    emit_block(
                        r_odd, x_allP[1], make_prep_quanta(r_odd + TBLK * BL, x_allP[0], "o")
                    )

    nc.finalize()
    return nc


def _make_runner(nc, n_cores):
    """Build a cached jitted executor for nc (compile once, reuse across
    kernel() calls). Mirrors bass2jax.run_bass_via_pjrt's multi-core path."""
    import jax
    from jax.sharding import Mesh, NamedSharding, PartitionSpec
    from jax.experimental.shard_map import shard_map
    from concourse import bass2jax, mybir
    from concourse.bass2jax import _bass_exec_p, partition_id_tensor

    bass2jax.install_neuronx_cc_hook()
    partition_name = nc.partition_id_tensor.name if nc.partition_id_tensor else None
    in_names, out_names, out_avals, zero_shapes = [], [], [], []
    for alloc in nc.m.functions[0].allocations:
        if not isinstance(alloc, mybir.MemoryLocationSet):
            continue
        name = alloc.memorylocations[0].name
        if alloc.kind == "ExternalInput":
            if name != partition_name:
                in_names.append(name)
        elif alloc.kind == "ExternalOutput":
            out_names.append(name)
            shape = tuple(alloc.tensor_shape)
            dtype = mybir.dt.np(alloc.dtype)
            out_avals.append(jax.core.ShapedArray(shape, dtype))
            zero_shapes.append((shape, dtype))
    n_params = len(in_names)
    n_outs = len(out_avals)
    all_in_names = list(in_names) + out_names
    if partition_name is not None:
        all_in_names.append(partition_name)
    donate = tuple(range(n_params, n_params + n_outs))

    def _body(*args_):
        operands = list(args_)
        if partition_name is not None:
            operands.append(partition_id_tensor())
        outs = _bass_exec_p.bind(
            *operands,
            out_avals=tuple(out_avals),
            in_names=tuple(all_in_names),
            out_names=tuple(out_names),
            lowering_input_output_aliases=(),
            sim_require_finite=True,
            sim_require_nnan=True,
            nc=nc,
        )
        return tuple(outs)

    devices = jax.devices()[:n_cores]
    mesh = Mesh(np.asarray(devices), ("core",))
    in_specs = (PartitionSpec("core"),) * (n_params + n_outs)
    out_specs = (PartitionSpec("core"),) * len(out_names)
    sharded = jax.jit(
        shard_map(
            _body, mesh=mesh, in_specs=in_specs, out_specs=out_specs, check_rep=False
        ),
        donate_argnums=donate,
        keep_unused=True,
    )
    sh = NamedSharding(mesh, PartitionSpec("core"))

    def run(in_maps):
        per_core = [[np.asarray(m[n]) for n in in_names] for m in in_maps]
        concat_in = [
            np.concatenate([per_core[c][i] for c in range(n_cores)], axis=0)
            for i in range(n_params)
        ]
        dev_in = [jax.device_put(a, sh) for a in concat_in]
        zs = [
            jax.device_put(np.zeros((n_cores * s[0], *s[1:]), d), sh)
            for (s, d) in zero_shapes
        ]
        outs = sharded(*dev_in, *zs)
        return [
            {
                name: np.asarray(outs[i]).reshape(n_cores, *out_avals[i].shape)[c]
                for i, name in enumerate(out_names)
            }
            for c in range(n_cores)
        ]

    return run


def kernel(x, Wz, Wh, Uz, Uh):
    from concourse.bass_utils import run_bass_kernel_spmd

    t_total = x.shape[0]
    hilo = os.environ.get("LGRU_HILO", "1") == "1"
    key = (t_total, hilo)
    if key not in _CACHED:
        _CACHED[key] = _build_nc(t_total, hilo=hilo)
    nc = _CACHED[key]

    x = np.ascontiguousarray(np.asarray(x, dtype=np.float32))
    Wz = np.ascontiguousarray(np.asarray(Wz, dtype=np.float32))
    Wh = np.ascontiguousarray(np.asarray(Wh, dtype=np.float32))
    Uz = np.ascontiguousarray(np.asarray(Uz, dtype=np.float32))
    Uh = np.ascontiguousarray(np.asarray(Uh, dtype=np.float32))

    xpad = np.zeros((t_total + TBLK, x.shape[1], x.shape[2]), np.float32)
    xpad[:t_total] = x

    in_maps = []
    for c in range(NCORES):
        in_maps.append(
            {
                "x": np.ascontiguousarray(xpad[:, c * BL : (c + 1) * BL, :]),
                "Wz": Wz,
                "Wh": Wh,
                "Uz": Uz,
                "Uh": Uh,
            }
        )

    trace = os.environ.get("LGRU_TRACE", "0") == "1"
    if trace:
        # traced path (works only where the NTFF hook exists)
        try:
            res = run_bass_kernel_spmd(
                nc, in_maps, core_ids=list(range(NCORES)), trace=True
            )
            if res.exec_time_ns is not None:
                print(f"HW exec time: {res.exec_time_ns} ns")
                kernel.last_exec_time_ns = res.exec_time_ns
                kernel.last_trace = res.instructions_and_trace
            return np.concatenate([r["hs"] for r in res.results], axis=1)
        except (ImportError, ModuleNotFoundError):
            pass
    rkey = ("runner", key)
    if rkey not in _CACHED:
        _CACHED[rkey] = _make_runner(nc, NCORES)
    results = _CACHED[rkey](in_maps)
    out = np.concatenate([r["hs"] for r in results], axis=1)
    return out
